# revision 1
# baseline (speedup 1.0000x reference)
"""Trainium2 Bass kernel for nn_AttentionNeNode (8-core SPMD).

Math being computed (see problem reference):
    sel  = inputs[:, in_idxs]            # [R, L] column gather
    qkv  = sel @ weights                 # [R, 3] -> q, k, v columns
    out  = sigmoid(softmax(q[-1] * k.T) @ v)   # only the LAST row's attention matters

Key transformations:
  1. Column gather + matmul == dense matmul with scattered weights:
         sel @ weights == inputs @ W_dense,
     where W_dense[f] = sum of weights[l] over l with in_idxs[l] == f.
     This turns random column access into a dense streaming read of `inputs`.
  2. Only row R-1's attention is needed, so each core computes k, v for its
     block of rows plus flash-softmax partial stats (max, sum_exp, sum_exp*v)
     per 512-row slice; host combines the 16 stat triples (the "unshard").
  3. `inputs` is pre-transposed/tiled on host so the contraction dim (F) lands
     on SBUF partitions and DMA descriptors are large contiguous runs.
  4. k and v come out of ONE m=2 fp32r matmul per chunk (rhs streamed once).
     v (psum partition 1) is moved to partition 0 via a tiny SBUF->SBUF DMA
     that overlaps the k-side softmax stats.
"""

import sys

if "/opt/trn_rl_repo" not in sys.path:
    sys.path.insert(0, "/opt/trn_rl_repo")

import numpy as np

import concourse.bacc as bacc
import concourse.tile as tile
from concourse import mybir
from concourse.bass_utils import run_bass_kernel_spmd

R, F = 8192, 4096
NCORES = 8
RB = R // NCORES            # 1024 rows per core
NSLICE = 2                  # row slices per core (one PSUM bank for [2, 512])
SLICE = RB // NSLICE        # 512
NCHUNK = F // 128           # 32 contraction chunks of 128
# chunks per DMA tile, per slice; the small final tile shrinks the exposed
# end-of-stream matmul burst (PE runs cold at 1.2 GHz in this DMA-bound kernel)
TILESPLITS = [[8, 8, 8, 8], [8, 8, 8, 6, 2]]
F32 = mybir.dt.float32
F32R = mybir.dt.float32r

_NC = None


def _build_nc():
    nc = bacc.Bacc("TRN2", target_bir_lowering=False, debug=False)
    xt = nc.dram_tensor("xt", [NSLICE, 128, NCHUNK, SLICE], F32R,
                        kind="ExternalInput").ap()
    wsb = nc.dram_tensor("wsb", [128, 3 * NCHUNK], F32R, kind="ExternalInput").ap()
    # last-row chunks duplicated x2: fp32r matmul needs moving free dim >= 2
    lrow = nc.dram_tensor("lrow", [128, 2 * NCHUNK], F32R,
                          kind="ExternalInput").ap()
    out = nc.dram_tensor("out", [1, 8], F32, kind="ExternalOutput").ap()

    AF = mybir.ActivationFunctionType
    ALU = mybir.AluOpType
    AX = mybir.AxisListType

    with tile.TileContext(nc) as tc:
        with tc.tile_pool(name="consts", bufs=1) as consts, \
             tc.tile_pool(name="xtiles", bufs=3) as xtiles, \
             tc.tile_pool(name="ps", bufs=2, space="PSUM") as psp, \
             tc.tile_pool(name="psq", bufs=1, space="PSUM") as psqp, \
             tc.tile_pool(name="tail", bufs=2) as tailp, \
             tc.tile_pool(name="fin", bufs=1) as finp:
            # the FIRST input tile dispatches before anything else so the
            # wire-limited 16 MiB stream starts as early as possible; the tiny
            # const loads follow on the same sync queue (they complete ~5 us
            # before the first matmul needs them). Keeping consts on the sync
            # queue matters: SDMA engines round-robin rings at packet
            # granularity, so a tiny transfer on another ring gets starved by
            # the tile stream and stalls the PE FIFO.
            x_first = xtiles.tile([128, TILESPLITS[0][0] * SLICE], F32R,
                                  tag="x_t")
            nc.sync.dma_start(out=x_first[:],
                              in_=xt[0, :, 0:TILESPLITS[0][0], :])
            w_t = consts.tile([128, 3 * NCHUNK], F32R)
            nc.sync.dma_start(out=w_t[:], in_=wsb)
            l_t = consts.tile([128, 2 * NCHUNK], F32R)
            nc.sync.dma_start(out=l_t[:], in_=lrow)

            ps_q = psqp.tile([1, 2], F32)
            qlast = finp.tile([1, 1], F32)
            outsb = finp.tile([1, 8], F32)
            nc.vector.memset(outsb[:], 0.0)

            for s in range(NSLICE):
                # one m=2 matmul per chunk makes [k; v] rows: stream rhs ONCE
                ps_kv = psp.tile([2, SLICE], F32, tag="ps_kv")
                c0 = 0
                for it, nt in enumerate(TILESPLITS[s]):
                    if s == 0 and it == 0:
                        x_t = x_first
                    else:
                        x_t = xtiles.tile([128, nt * SLICE], F32R, tag="x_t")
                        nc.sync.dma_start(out=x_t[:],
                                          in_=xt[s, :, c0:c0 + nt, :])
                    for u in range(nt):
                        c = c0 + u
                        rhs = x_t[:, u * SLICE:(u + 1) * SLICE]
                        st, sp = (c == 0), (c == NCHUNK - 1)
                        nc.tensor.matmul(ps_kv[:], w_t[:, 3 * c + 1:3 * c + 3],
                                         rhs, start=st, stop=sp)
                        if s == 0:
                            nc.tensor.matmul(ps_q[:], w_t[:, 3 * c:3 * c + 1],
                                             l_t[:, 2 * c:2 * c + 2],
                                             start=st, stop=sp)
                    c0 += nt
                if s == 0:
                    nc.scalar.copy(out=qlast[:], in_=ps_q[:, 0:1])
                # evacuate PSUM; k stays on partition 0 (directly usable),
                # v (partition 1) is flattened down via a small SBUF->SBUF DMA
                # that overlaps the k-side stats chain below
                kv_sb = tailp.tile([2, SLICE], F32, tag="kv_sb")
                nc.scalar.copy(out=kv_sb[:], in_=ps_kv[:])
                v_f = tailp.tile([1, SLICE], F32, tag="v_f")
                nc.sync.dma_start(out=v_f[:], in_=kv_sb[1:2, :])
                # flash-softmax partial stats for this slice of 512 rows;
                # logits reads k straight from PSUM partition 0 so the k-side
                # chain runs concurrently with the kv copy + v flatten DMA
                logits = tailp.tile([1, SLICE], F32, tag="logits")
                nc.vector.tensor_scalar_mul(out=logits[:], in0=ps_kv[0:1, :],
                                            scalar1=qlast[:])
                nc.vector.tensor_reduce(out=outsb[:, 3 * s:3 * s + 1],
                                        in_=logits[:], axis=AX.X, op=ALU.max,
                                        negate=True)
                e_t = tailp.tile([1, SLICE], F32, tag="e_t")
                nc.scalar.activation(out=e_t[:], in_=logits[:], func=AF.Exp,
                                     bias=outsb[:, 3 * s:3 * s + 1], scale=1.0,
                                     accum_out=outsb[:, 3 * s + 1:3 * s + 2])
                # fused (e*1)*v with accumulator: one native DVE instruction
                # replaces tensor_mul + reduce_sum on the exposed tail
                scr = tailp.tile([1, SLICE], F32, tag="scr")
                nc.vector.scalar_tensor_tensor(
                    out=scr[:], in0=e_t[:], scalar=1.0, in1=v_f[:],
                    op0=ALU.mult, op1=ALU.mult,
                    accum_out=outsb[:, 3 * s + 2:3 * s + 3])

            nc.sync.dma_start(out=out, in_=outsb[:])
    nc.finalize()
    return nc


def _get_nc():
    global _NC
    if _NC is None:
        _NC = _build_nc()
    return _NC


def _prep_inputs(inputs, in_idxs, weights):
    inputs = np.ascontiguousarray(np.asarray(inputs, dtype=np.float32))
    idx = np.asarray(in_idxs).astype(np.int64)
    w = np.asarray(weights, dtype=np.float32)

    # scatter-add weights into dense [F, 3]: sel @ weights == inputs @ wd
    wd = np.zeros((F, 3), dtype=np.float32)
    np.add.at(wd, idx, w)
    # SBUF layout [128, 3*NCHUNK]: wsb[p, 3c+j] = wd[c*128+p, j]
    wsb = np.ascontiguousarray(
        wd.reshape(NCHUNK, 128, 3).transpose(1, 0, 2).reshape(128, 3 * NCHUNK))
    # last row of inputs, chunked + duplicated: lrow[p, 2c+{0,1}] = x[R-1, c*128+p]
    lrow = np.ascontiguousarray(
        np.repeat(inputs[R - 1].reshape(NCHUNK, 128).T, 2, axis=1))

    # xt[core][s, p, c, col] = inputs[core*RB + s*SLICE + col, c*128 + p]
    x5 = inputs.reshape(NCORES, NSLICE, SLICE, NCHUNK, 128)
    xt_all = np.ascontiguousarray(x5.transpose(0, 1, 4, 3, 2))

    return [{"xt": xt_all[i], "wsb": wsb, "lrow": lrow} for i in range(NCORES)]


def _combine(outs):
    # outs: [N, 8]: per slice s in {0,1}: (-max_logit, sum_exp, sum_exp_v) at
    # columns 3s..3s+2. Exact flash-softmax combine in fp64 on the host.
    o = np.asarray(outs, dtype=np.float64)
    trip = np.concatenate([o[:, 0:3], o[:, 3:6]], axis=0)
    m = -trip[:, 0]
    s = trip[:, 1]
    w = trip[:, 2]
    mx = m.max()
    scale = np.exp(m - mx)
    val = (w * scale).sum() / (s * scale).sum()
    if val >= 0:
        sig = 1.0 / (1.0 + np.exp(-val))
    else:
        ev = np.exp(val)
        sig = ev / (1.0 + ev)
    return np.array([[sig]], dtype=np.float32)


def kernel(inputs, in_idxs, weights):
    nc = _get_nc()
    in_maps = _prep_inputs(inputs, in_idxs, weights)
    res = run_bass_kernel_spmd(nc, in_maps, core_ids=list(range(NCORES)))
    outs = np.stack([res.results[i]["out"][0] for i in range(NCORES)])
    return _combine(outs)


if __name__ == "__main__":
    rng = np.random.default_rng(0)
    inputs = rng.standard_normal((R, F), dtype=np.float32)
    in_idxs = rng.integers(0, F, size=2048)
    weights = rng.standard_normal((2048, 3), dtype=np.float32)
    got = kernel(inputs, in_idxs, weights)
    sel = inputs[:, in_idxs]
    qkv = sel.astype(np.float64) @ weights.astype(np.float64)
    q, k, v = qkv[:, 0], qkv[:, 1], qkv[:, 2]
    logits = q[-1] * k
    a = np.exp(logits - logits.max())
    want = a @ v / a.sum()
    want = 1.0 / (1.0 + np.exp(-want))
    print("got", got, "want", want, "relerr", abs(got[0, 0] - want) / max(abs(want), 1e-30))



# revision 2
# speedup vs baseline: 2.8580x; 2.8580x over previous
"""Trainium2 Bass kernel for nn_AttentionNeNode (8-core SPMD).

Math being computed (see problem reference):
    sel  = inputs[:, in_idxs]            # [R, L] column gather
    qkv  = sel @ weights                 # [R, 3] -> q, k, v columns
    out  = sigmoid(softmax(q[-1] * k.T) @ v)   # only the LAST row's attention matters

Key transformations:
  1. Column gather + matmul == dense matmul with scattered weights:
         sel @ weights == inputs @ W_dense,
     where W_dense[f] = sum of weights[l] over l with in_idxs[l] == f.
  2. Only the UNIQUE gathered columns matter (W_dense is zero elsewhere), so
     the host packs just those columns (~1620 of 4096) before shipping to the
     device: 2.5x less HBM traffic, numerically exact.
  3. The packed activations stream in fp8e4m3 (4x fewer bytes than f32). The
     device computes approximate k,v per row; the host then does the exact
     flash-softmax combine in f64, re-computing k,v from the original f32
     data for the handful of rows whose logits are within a safety margin of
     the max (softmax is a near-delta here), so device precision cannot
     affect the final answer.
  4. inputs are pre-transposed/tiled on host so the contraction dim lands on
     SBUF partitions and DMA descriptors are large contiguous runs.
  5. k and v come out of ONE m=2 matmul per 128-feature chunk (rhs streamed
     once); raw k,v rows ship back to the host per row-slice.
"""

import sys

if "/opt/trn_rl_repo" not in sys.path:
    sys.path.insert(0, "/opt/trn_rl_repo")

import numpy as np
import ml_dtypes

import concourse.bacc as bacc
import concourse.tile as tile
from concourse import mybir
from concourse.bass_utils import run_bass_kernel_spmd

R, F = 8192, 4096
NCORES = 8
RB = R // NCORES            # 1024 rows per core
NSLICE = 2                  # row slices per core (one PSUM bank per [2, 512])
SLICE = RB // NSLICE        # 512
FP8 = mybir.dt.float8e4
F32 = mybir.dt.float32
NP_FP8 = ml_dtypes.float8_e4m3
# margin (in logit units) below the max logit within which rows are exactly
# re-computed on the host. fp8 k-error std is ~1.0 -> logit error std
# ~|q|*1.0; margin 40*|q| + 40 covers >30 sigma while keeping the candidate
# set tiny for well-separated maxima.
CAND_MARGIN_Q = 40.0

_NC_CACHE = {}


def _tile_splits(pch):
    # DMA tiles per slice, in chunks: keep each tile a few hundred KB so the
    # stream pipelines against the PE without descriptor overhead dominating.
    splits = []
    rem = pch
    while rem > 0:
        t = min(5, rem)
        splits.append(t)
        rem -= t
    return splits


def _build_nc(pch):
    nc = bacc.Bacc("TRN2", target_bir_lowering=False, debug=False)
    xt = nc.dram_tensor("xt", [NSLICE, 128, pch, SLICE], FP8,
                        kind="ExternalInput").ap()
    wsb = nc.dram_tensor("wsb", [128, 2 * pch], FP8, kind="ExternalInput").ap()
    out = nc.dram_tensor("out", [2, RB], F32, kind="ExternalOutput").ap()

    splits = _tile_splits(pch)

    with tile.TileContext(nc) as tc:
        with tc.tile_pool(name="consts", bufs=1) as consts, \
             tc.tile_pool(name="xtiles", bufs=3) as xtiles, \
             tc.tile_pool(name="ps", bufs=2, space="PSUM") as psp, \
             tc.tile_pool(name="tail", bufs=2) as tailp:
            # first x tile dispatches before anything else so the stream
            # starts as early as possible; the tiny weight load follows on
            # the same sync queue and lands right behind it.
            x_first = xtiles.tile([128, splits[0] * SLICE], FP8, tag="x_t")
            nc.sync.dma_start(out=x_first[:], in_=xt[0, :, 0:splits[0], :])
            w_t = consts.tile([128, 2 * pch], FP8)
            nc.sync.dma_start(out=w_t[:], in_=wsb)

            for s in range(NSLICE):
                ps_kv = psp.tile([2, SLICE], F32, tag="ps_kv")
                c0 = 0
                for it, nt in enumerate(splits):
                    if s == 0 and it == 0:
                        x_t = x_first
                    else:
                        x_t = xtiles.tile([128, nt * SLICE], FP8, tag="x_t")
                        nc.sync.dma_start(out=x_t[:],
                                          in_=xt[s, :, c0:c0 + nt, :])
                    for u in range(nt):
                        c = c0 + u
                        rhs = x_t[:, u * SLICE:(u + 1) * SLICE]
                        st, sp = (c == 0), (c == pch - 1)
                        nc.tensor.matmul(ps_kv[:], w_t[:, 2 * c:2 * c + 2],
                                         rhs, start=st, stop=sp)
                    c0 += nt
                # evacuate PSUM and ship the raw k,v rows for this slice
                kv_sb = tailp.tile([2, SLICE], F32, tag="kv_sb")
                nc.scalar.copy(out=kv_sb[:], in_=ps_kv[:])
                nc.sync.dma_start(out=out[:, s * SLICE:(s + 1) * SLICE],
                                  in_=kv_sb[:])
    nc.finalize()
    return nc


def _get_nc(pch):
    if pch not in _NC_CACHE:
        _NC_CACHE[pch] = _build_nc(pch)
    return _NC_CACHE[pch]


def _prep_inputs(inputs, in_idxs, weights):
    inputs = np.ascontiguousarray(np.asarray(inputs, dtype=np.float32))
    idx = np.asarray(in_idxs).astype(np.int64)
    w = np.asarray(weights, dtype=np.float32)

    # scatter-add weights onto the UNIQUE gathered columns:
    # sel @ weights == inputs[:, uniq] @ wu
    uniq, inv = np.unique(idx, return_inverse=True)
    nu = len(uniq)
    wu = np.zeros((nu, 3), dtype=np.float64)
    np.add.at(wu, inv, w.astype(np.float64))

    pch = (nu + 127) // 128
    fpad = pch * 128

    # packed activation block [R, fpad] in fp8 (zero-padded features)
    a = np.zeros((R, fpad), dtype=NP_FP8)
    a[:, :nu] = inputs[:, uniq].astype(NP_FP8)
    wpad = np.zeros((fpad, 3), dtype=np.float64)
    wpad[:nu] = wu

    # wsb[p, 2c+j] = wpad[c*128+p, 1+j]  (k and v weight columns, fp8)
    wsb = np.ascontiguousarray(
        wpad[:, 1:3].astype(np.float32).astype(NP_FP8)
        .reshape(pch, 128, 2).transpose(1, 0, 2).reshape(128, 2 * pch))

    # xt[core][s, p, c, col] = a[core*RB + s*SLICE + col, c*128 + p]
    x5 = a.reshape(NCORES, NSLICE, SLICE, pch, 128)
    xt_all = np.ascontiguousarray(x5.transpose(0, 1, 4, 3, 2))

    in_maps = [{"xt": xt_all[i], "wsb": wsb} for i in range(NCORES)]
    host_ctx = {
        "inputs": inputs, "uniq": uniq, "wu": wu, "pch": pch,
        # exact last-row q in f64 (one tiny dot product)
        "q_last": float(inputs[R - 1, uniq].astype(np.float64) @ wu[:, 0]),
    }
    return in_maps, host_ctx


def _combine(kv, host_ctx):
    # kv: [NCORES, 2, RB] device k,v (fp8-accurate). Exact f64 flash-softmax
    # with host-side exact recompute of every row whose logit is within the
    # safety margin of the max.
    k_dev = np.asarray(kv, dtype=np.float64)[:, 0, :].reshape(R)
    v_dev = np.asarray(kv, dtype=np.float64)[:, 1, :].reshape(R)
    q = host_ctx["q_last"]
    x = q * k_dev
    margin = CAND_MARGIN_Q * max(abs(q), 1.0) + 40.0
    cand = np.nonzero(x >= x.max() - margin)[0]
    # exact k,v for candidate rows from the original f32 data
    a_c = host_ctx["inputs"][cand][:, host_ctx["uniq"]].astype(np.float64)
    kv_c = a_c @ host_ctx["wu"][:, 1:3]
    x_c = q * kv_c[:, 0]
    v_c = kv_c[:, 1]
    x[cand] = x_c
    v = v_dev
    v[cand] = v_c
    m = x.max()
    e = np.exp(x - m)
    val = (e * v).sum() / e.sum()
    if val >= 0:
        sig = 1.0 / (1.0 + np.exp(-val))
    else:
        ev = np.exp(val)
        sig = ev / (1.0 + ev)
    return np.array([[sig]], dtype=np.float32)


def kernel(inputs, in_idxs, weights):
    in_maps, host_ctx = _prep_inputs(inputs, in_idxs, weights)
    nc = _get_nc(host_ctx["pch"])
    res = run_bass_kernel_spmd(nc, in_maps, core_ids=list(range(NCORES)))
    kv = np.stack([res.results[i]["out"] for i in range(NCORES)])
    return _combine(kv, host_ctx)


if __name__ == "__main__":
    rng = np.random.default_rng(0)
    inputs = rng.standard_normal((R, F), dtype=np.float32)
    in_idxs = rng.integers(0, F, size=2048)
    weights = rng.standard_normal((2048, 3), dtype=np.float32)
    got = kernel(inputs, in_idxs, weights)
    sel = inputs[:, in_idxs]
    qkv = sel.astype(np.float64) @ weights.astype(np.float64)
    q, k, v = qkv[:, 0], qkv[:, 1], qkv[:, 2]
    logits = q[-1] * k
    a = np.exp(logits - logits.max())
    want = a @ v / a.sum()
    want = 1.0 / (1.0 + np.exp(-want))
    print("got", got, "want", want,
          "relerr", abs(got[0, 0] - want) / max(abs(want), 1e-30))


# revision 5
# speedup vs baseline: 3.0801x; 1.0777x over previous
"""Trainium2 Bass kernel for nn_AttentionNeNode (8-core SPMD).

Math being computed (see problem reference):
    sel  = inputs[:, in_idxs]            # [R, L] column gather
    qkv  = sel @ weights                 # [R, 3] -> q, k, v columns
    out  = sigmoid(softmax(q[-1] * k.T) @ v)   # only the LAST row's attention matters

Key transformations:
  1. Column gather + matmul == dense matmul with scattered weights:
         sel @ weights == inputs @ W_dense,
     where W_dense[f] = sum of weights[l] over l with in_idxs[l] == f.
  2. Only the UNIQUE gathered columns matter (W_dense is zero elsewhere), so
     the host packs just those columns (~1620 of 4096) before shipping to the
     device: 2.5x less HBM traffic, numerically exact.
  3. The packed activations stream in fp8e4m3 (4x fewer bytes than f32), and
     matmuls run in DoubleRow perf mode (two 128-feature chunks per pass) so
     the PE keeps up with the DMA stream.
  4. The device computes k,v per row; the host does the exact flash-softmax
     combine in f64, re-computing k,v from the original f32 data for the
     handful of rows whose logits are within a safety margin of the max, so
     device precision cannot affect the final answer.
  5. inputs are pre-transposed/tiled on host so the contraction dim lands on
     SBUF partitions and DMA descriptors are large contiguous runs.
"""

import sys

if "/opt/trn_rl_repo" not in sys.path:
    sys.path.insert(0, "/opt/trn_rl_repo")

import numpy as np
import ml_dtypes

import concourse.bacc as bacc
import concourse.tile as tile
from concourse import mybir
from concourse.bass_utils import run_bass_kernel_spmd

R, F = 8192, 4096
NCORES = 8
RB = R // NCORES            # 1024 rows per core
NSLICE = 2                  # row slices per core (one PSUM bank per [2, 512])
SLICE = RB // NSLICE        # 512
FP8 = mybir.dt.float8e4
F32 = mybir.dt.float32
NP_FP8 = ml_dtypes.float8_e4m3
DR = mybir.MatmulPerfMode.DoubleRow
# margin (in logit units) below the max logit within which rows are exactly
# re-computed on the host; fp8 logit error std is ~|q| so this covers >30
# sigma while keeping the candidate set tiny for well-separated maxima.
CAND_MARGIN_Q = 40.0

_NC_CACHE = {}


def _build_nc(pairs):
    nc = bacc.Bacc("TRN2", target_bir_lowering=False, debug=False)
    xt = nc.dram_tensor("xt", [NSLICE, 128, pairs, 2, SLICE], FP8,
                        kind="ExternalInput").ap()
    # the two k-tile half-weights must sit >=16B apart in SBUF for DoubleRow
    # LDWEIGHTS (s3_lw dual-fp8 restriction), hence the padded last dim
    wsb = nc.dram_tensor("wsb", [128, pairs, 2, 16], FP8,
                         kind="ExternalInput").ap()
    out = nc.dram_tensor("out", [2, RB], F32, kind="ExternalOutput").ap()

    # pair-granularity DMA tiles per slice: two tiles so the PE can start on
    # the first half of a slice while the second half streams
    ta = (pairs + 1) // 2
    splits = [ta, pairs - ta]

    with tile.TileContext(nc) as tc:
        with tc.tile_pool(name="consts", bufs=1) as consts, \
             tc.tile_pool(name="xtiles", bufs=3) as xtiles, \
             tc.tile_pool(name="ps", bufs=2, space="PSUM") as psp, \
             tc.tile_pool(name="tail", bufs=2) as tailp:
            # first x tile dispatches before anything else so the stream
            # starts as early as possible; the tiny weight load rides the
            # scalar HWDGE ring in parallel.
            x_first = xtiles.tile([128, splits[0], 2, SLICE], FP8, tag="x_t")
            nc.sync.dma_start(out=x_first[:], in_=xt[0, :, 0:splits[0], :, :])
            w_t = consts.tile([128, pairs, 2, 16], FP8)
            nc.scalar.dma_start(out=w_t[:], in_=wsb)

            for s in range(NSLICE):
                ps_kv = psp.tile([2, SLICE], F32, tag="ps_kv")
                p0 = 0
                for it, npair in enumerate(splits):
                    if s == 0 and it == 0:
                        x_t = x_first
                    else:
                        x_t = xtiles.tile([128, npair, 2, SLICE], FP8,
                                          tag="x_t")
                        nc.sync.dma_start(out=x_t[:],
                                          in_=xt[s, :, p0:p0 + npair, :, :])
                    for u in range(npair):
                        p = p0 + u
                        st, sp = (p == 0), (p == pairs - 1)
                        nc.tensor.matmul(ps_kv[:], w_t[:, p, :, 0:2],
                                         x_t[:, u, :, :], start=st, stop=sp,
                                         perf_mode=DR)
                    p0 += npair
                # evacuate PSUM and ship the raw k,v rows for this slice
                kv_sb = tailp.tile([2, SLICE], F32, tag="kv_sb")
                nc.scalar.copy(out=kv_sb[:], in_=ps_kv[:])
                nc.scalar.dma_start(out=out[:, s * SLICE:(s + 1) * SLICE],
                                    in_=kv_sb[:])
    nc.finalize()
    return nc


def _get_nc(pairs):
    if pairs not in _NC_CACHE:
        _NC_CACHE[pairs] = _build_nc(pairs)
    return _NC_CACHE[pairs]


def _prep_inputs(inputs, in_idxs, weights):
    inputs = np.ascontiguousarray(np.asarray(inputs, dtype=np.float32))
    idx = np.asarray(in_idxs).astype(np.int64)
    w = np.asarray(weights, dtype=np.float32)

    # scatter-add weights onto the UNIQUE gathered columns:
    # sel @ weights == inputs[:, uniq] @ wu
    uniq, inv = np.unique(idx, return_inverse=True)
    nu = len(uniq)
    wu = np.zeros((nu, 3), dtype=np.float64)
    np.add.at(wu, inv, w.astype(np.float64))

    pairs = (nu + 255) // 256
    fpad = pairs * 256

    # packed activation block [R, fpad] in fp8 (zero-padded features)
    a = np.zeros((R, fpad), dtype=NP_FP8)
    a[:, :nu] = inputs[:, uniq].astype(NP_FP8)
    wpad = np.zeros((fpad, 3), dtype=np.float64)
    wpad[:nu] = wu

    # wsb[p, P, i, m] = wpad[(2P+i)*128 + p, 1+m]  (k and v weight columns),
    # padded to 16 fp8 slots per (P, i) for the DoubleRow LDWEIGHTS step rule
    wsb = np.zeros((128, pairs, 2, 16), dtype=NP_FP8)
    wsb[:, :, :, 0:2] = (
        wpad[:, 1:3].astype(np.float32).astype(NP_FP8)
        .reshape(pairs, 2, 128, 2).transpose(2, 0, 1, 3))

    # xt[core][s, p, P, i, col] = a[core*RB + s*SLICE + col, (2P+i)*128 + p]
    x6 = a.reshape(NCORES, NSLICE, SLICE, pairs, 2, 128)
    xt_all = np.ascontiguousarray(x6.transpose(0, 1, 5, 3, 4, 2))

    in_maps = [{"xt": xt_all[i], "wsb": wsb} for i in range(NCORES)]
    host_ctx = {
        "inputs": inputs, "uniq": uniq, "wu": wu, "pairs": pairs,
        # exact last-row q in f64 (one tiny dot product)
        "q_last": float(inputs[R - 1, uniq].astype(np.float64) @ wu[:, 0]),
    }
    return in_maps, host_ctx


def _combine(kv, host_ctx):
    # kv: [NCORES, 2, RB] device k,v (fp8-accurate). Exact f64 flash-softmax
    # with host-side exact recompute of every row whose logit is within the
    # safety margin of the max.
    k_dev = np.asarray(kv, dtype=np.float64)[:, 0, :].reshape(R)
    v_dev = np.asarray(kv, dtype=np.float64)[:, 1, :].reshape(R)
    q = host_ctx["q_last"]
    x = q * k_dev
    margin = CAND_MARGIN_Q * max(abs(q), 1.0) + 40.0
    cand = np.nonzero(x >= x.max() - margin)[0]
    # exact k,v for candidate rows from the original f32 data
    a_c = host_ctx["inputs"][cand][:, host_ctx["uniq"]].astype(np.float64)
    kv_c = a_c @ host_ctx["wu"][:, 1:3]
    x[cand] = q * kv_c[:, 0]
    v = v_dev
    v[cand] = kv_c[:, 1]
    m = x.max()
    e = np.exp(x - m)
    val = (e * v).sum() / e.sum()
    if val >= 0:
        sig = 1.0 / (1.0 + np.exp(-val))
    else:
        ev = np.exp(val)
        sig = ev / (1.0 + ev)
    return np.array([[sig]], dtype=np.float32)


def kernel(inputs, in_idxs, weights):
    in_maps, host_ctx = _prep_inputs(inputs, in_idxs, weights)
    nc = _get_nc(host_ctx["pairs"])
    res = run_bass_kernel_spmd(nc, in_maps, core_ids=list(range(NCORES)))
    kv = np.stack([res.results[i]["out"] for i in range(NCORES)])
    return _combine(kv, host_ctx)


if __name__ == "__main__":
    rng = np.random.default_rng(0)
    inputs = rng.standard_normal((R, F), dtype=np.float32)
    in_idxs = rng.integers(0, F, size=2048)
    weights = rng.standard_normal((2048, 3), dtype=np.float32)
    got = kernel(inputs, in_idxs, weights)
    sel = inputs[:, in_idxs]
    qkv = sel.astype(np.float64) @ weights.astype(np.float64)
    q, k, v = qkv[:, 0], qkv[:, 1], qkv[:, 2]
    logits = q[-1] * k
    a = np.exp(logits - logits.max())
    want = a @ v / a.sum()
    want = 1.0 / (1.0 + np.exp(-want))
    print("got", got, "want", want,
          "relerr", abs(got[0, 0] - want) / max(abs(want), 1e-30))


# revision 7
# speedup vs baseline: 3.2092x; 1.0419x over previous
"""Trainium2 Bass kernel for nn_AttentionNeNode (8-core SPMD).

Math being computed (see problem reference):
    sel  = inputs[:, in_idxs]            # [R, L] column gather
    qkv  = sel @ weights                 # [R, 3] -> q, k, v columns
    out  = sigmoid(softmax(q[-1] * k.T) @ v)   # only the LAST row's attention matters

Key transformations:
  1. Column gather + matmul == dense matmul with scattered weights:
         sel @ weights == inputs @ W_dense,
     where W_dense[f] = sum of weights[l] over l with in_idxs[l] == f.
  2. Only the UNIQUE gathered columns matter (W_dense is zero elsewhere), so
     the host packs just those columns (~1620 of 4096) before shipping to the
     device: 2.5x less HBM traffic, numerically exact.
  3. The packed activations stream in fp8e4m3 (4x fewer bytes than f32), and
     matmuls run in DoubleRow perf mode (two 128-feature chunks per pass) so
     the PE keeps up with the DMA stream.
  4. The device computes k,v per row; the host does the exact flash-softmax
     combine in f64, re-computing k,v from the original f32 data for the
     handful of rows whose logits are within a safety margin of the max, so
     device precision cannot affect the final answer.
  5. inputs are pre-transposed/tiled on host so the contraction dim lands on
     SBUF partitions and DMA descriptors are large contiguous runs.
"""

import sys

if "/opt/trn_rl_repo" not in sys.path:
    sys.path.insert(0, "/opt/trn_rl_repo")

import numpy as np
import ml_dtypes

import concourse.bacc as bacc
import concourse.tile as tile
from concourse import mybir
from concourse.bass_utils import run_bass_kernel_spmd

R, F = 8192, 4096
NCORES = 8
RB = R // NCORES            # 1024 rows per core
NSLICE = 2                  # row slices per core (one PSUM bank per [2, 512])
SLICE = RB // NSLICE        # 512
FP8 = mybir.dt.float8e4
F32 = mybir.dt.float32
NP_FP8 = ml_dtypes.float8_e4m3
DR = mybir.MatmulPerfMode.DoubleRow
# margin (in logit units) below the max logit within which rows are exactly
# re-computed on the host; fp8 logit error std is ~|q| so this covers >30
# sigma while keeping the candidate set tiny for well-separated maxima.
CAND_MARGIN_Q = 40.0

_NC_CACHE = {}


def _build_nc(pairs):
    nc = bacc.Bacc("TRN2", target_bir_lowering=False, debug=False)
    xt = nc.dram_tensor("xt", [NSLICE, 128, pairs, 2, SLICE], FP8,
                        kind="ExternalInput").ap()
    # the two k-tile half-weights must sit >=16B apart in SBUF for DoubleRow
    # LDWEIGHTS (s3_lw dual-fp8 restriction), hence the padded last dim
    wsb = nc.dram_tensor("wsb", [128, pairs, 2, 16], FP8,
                         kind="ExternalInput").ap()
    out = nc.dram_tensor("out", [2, RB], F32, kind="ExternalOutput").ap()

    # pair-granularity DMA tiles per slice. The final tile of the final slice
    # is a single pair so almost no matmul work remains once the last byte
    # lands (tile-granularity semaphores gate the PE on whole tiles).
    def tiles_of(n, last_small):
        out = []
        if last_small and n > 1:
            n -= 1
        while n > 0:
            t = min(4, n)
            out.append(t)
            n -= t
        if last_small:
            out.append(1)
        return out

    slice_splits = [tiles_of(pairs, s == NSLICE - 1) for s in range(NSLICE)]

    with tile.TileContext(nc) as tc:
        nbuf = sum(len(sp) for sp in slice_splits)
        with tc.tile_pool(name="consts", bufs=1) as consts, \
             tc.tile_pool(name="xtiles", bufs=nbuf) as xtiles, \
             tc.tile_pool(name="ps", bufs=2, space="PSUM") as psp, \
             tc.tile_pool(name="tail", bufs=2) as tailp:
            # first x tile dispatches before anything else so the stream
            # starts as early as possible; the tiny weight load rides the
            # scalar HWDGE ring in parallel. Enough tile bufs to hold every
            # x tile at once, so all stream DMAs issue back-to-back.
            sp0 = slice_splits[0][0]
            x_first = xtiles.tile([128, sp0, 2, SLICE], FP8, tag="x_t")
            nc.sync.dma_start(out=x_first[:], in_=xt[0, :, 0:sp0, :, :])
            w_t = consts.tile([128, pairs, 2, 16], FP8)
            nc.scalar.dma_start(out=w_t[:], in_=wsb)

            for s in range(NSLICE):
                ps_kv = psp.tile([2, SLICE], F32, tag="ps_kv")
                p0 = 0
                for it, npair in enumerate(slice_splits[s]):
                    if s == 0 and it == 0:
                        x_t = x_first
                    else:
                        x_t = xtiles.tile([128, npair, 2, SLICE], FP8,
                                          tag="x_t")
                        nc.sync.dma_start(out=x_t[:],
                                          in_=xt[s, :, p0:p0 + npair, :, :])
                    for u in range(npair):
                        p = p0 + u
                        st, sp = (p == 0), (p == pairs - 1)
                        nc.tensor.matmul(ps_kv[:], w_t[:, p, :, 0:2],
                                         x_t[:, u, :, :], start=st, stop=sp,
                                         perf_mode=DR)
                    p0 += npair
                # evacuate PSUM and ship the raw k,v rows for this slice
                kv_sb = tailp.tile([2, SLICE], F32, tag="kv_sb")
                nc.scalar.copy(out=kv_sb[:], in_=ps_kv[:])
                nc.scalar.dma_start(out=out[:, s * SLICE:(s + 1) * SLICE],
                                    in_=kv_sb[:])
    nc.finalize()
    return nc


def _get_nc(pairs):
    if pairs not in _NC_CACHE:
        _NC_CACHE[pairs] = _build_nc(pairs)
    return _NC_CACHE[pairs]


def _prep_inputs(inputs, in_idxs, weights):
    inputs = np.ascontiguousarray(np.asarray(inputs, dtype=np.float32))
    idx = np.asarray(in_idxs).astype(np.int64)
    w = np.asarray(weights, dtype=np.float32)

    # scatter-add weights onto the UNIQUE gathered columns:
    # sel @ weights == inputs[:, uniq] @ wu
    uniq, inv = np.unique(idx, return_inverse=True)
    nu = len(uniq)
    wu = np.zeros((nu, 3), dtype=np.float64)
    np.add.at(wu, inv, w.astype(np.float64))

    pairs = (nu + 255) // 256
    fpad = pairs * 256

    # packed activation block [R, fpad] in fp8 (zero-padded features)
    a = np.zeros((R, fpad), dtype=NP_FP8)
    a[:, :nu] = inputs[:, uniq].astype(NP_FP8)
    wpad = np.zeros((fpad, 3), dtype=np.float64)
    wpad[:nu] = wu

    # wsb[p, P, i, m] = wpad[(2P+i)*128 + p, 1+m]  (k and v weight columns),
    # padded to 16 fp8 slots per (P, i) for the DoubleRow LDWEIGHTS step rule
    wsb = np.zeros((128, pairs, 2, 16), dtype=NP_FP8)
    wsb[:, :, :, 0:2] = (
        wpad[:, 1:3].astype(np.float32).astype(NP_FP8)
        .reshape(pairs, 2, 128, 2).transpose(2, 0, 1, 3))

    # xt[core][s, p, P, i, col] = a[core*RB + s*SLICE + col, (2P+i)*128 + p]
    x6 = a.reshape(NCORES, NSLICE, SLICE, pairs, 2, 128)
    xt_all = np.ascontiguousarray(x6.transpose(0, 1, 5, 3, 4, 2))

    in_maps = [{"xt": xt_all[i], "wsb": wsb} for i in range(NCORES)]
    host_ctx = {
        "inputs": inputs, "uniq": uniq, "wu": wu, "pairs": pairs,
        # exact last-row q in f64 (one tiny dot product)
        "q_last": float(inputs[R - 1, uniq].astype(np.float64) @ wu[:, 0]),
    }
    return in_maps, host_ctx


def _combine(kv, host_ctx):
    # kv: [NCORES, 2, RB] device k,v (fp8-accurate). Exact f64 flash-softmax
    # with host-side exact recompute of every row whose logit is within the
    # safety margin of the max.
    k_dev = np.asarray(kv, dtype=np.float64)[:, 0, :].reshape(R)
    v_dev = np.asarray(kv, dtype=np.float64)[:, 1, :].reshape(R)
    q = host_ctx["q_last"]
    x = q * k_dev
    margin = CAND_MARGIN_Q * max(abs(q), 1.0) + 40.0
    cand = np.nonzero(x >= x.max() - margin)[0]
    # exact k,v for candidate rows from the original f32 data
    a_c = host_ctx["inputs"][cand][:, host_ctx["uniq"]].astype(np.float64)
    kv_c = a_c @ host_ctx["wu"][:, 1:3]
    x[cand] = q * kv_c[:, 0]
    v = v_dev
    v[cand] = kv_c[:, 1]
    m = x.max()
    e = np.exp(x - m)
    val = (e * v).sum() / e.sum()
    if val >= 0:
        sig = 1.0 / (1.0 + np.exp(-val))
    else:
        ev = np.exp(val)
        sig = ev / (1.0 + ev)
    return np.array([[sig]], dtype=np.float32)


def kernel(inputs, in_idxs, weights):
    in_maps, host_ctx = _prep_inputs(inputs, in_idxs, weights)
    nc = _get_nc(host_ctx["pairs"])
    res = run_bass_kernel_spmd(nc, in_maps, core_ids=list(range(NCORES)))
    kv = np.stack([res.results[i]["out"] for i in range(NCORES)])
    return _combine(kv, host_ctx)


if __name__ == "__main__":
    rng = np.random.default_rng(0)
    inputs = rng.standard_normal((R, F), dtype=np.float32)
    in_idxs = rng.integers(0, F, size=2048)
    weights = rng.standard_normal((2048, 3), dtype=np.float32)
    got = kernel(inputs, in_idxs, weights)
    sel = inputs[:, in_idxs]
    qkv = sel.astype(np.float64) @ weights.astype(np.float64)
    q, k, v = qkv[:, 0], qkv[:, 1], qkv[:, 2]
    logits = q[-1] * k
    a = np.exp(logits - logits.max())
    want = a @ v / a.sum()
    want = 1.0 / (1.0 + np.exp(-want))
    print("got", got, "want", want,
          "relerr", abs(got[0, 0] - want) / max(abs(want), 1e-30))
